# revision 24
# baseline (speedup 1.0000x reference)
"""DenseGAT Trainium2 kernel (8 NeuronCores, batch-parallel).

Math: per (batch, head):
  h = x @ W.T ; a_src[i] = h[i]*att_src ; a_dst[j] = h[j]*att_dst
  s_ij = a_src[i] + a_dst[j] ; P = adj * exp(leakyrelu_0.2(s))
  out[i] = (P @ h)[i] / sum_j P[i,j]

Key identity: exp(lrelu(s)) = [s>=0]*u_i*v_j + [s<0]*p_i*q_j with
  u = exp(a_src), v = exp(a_dst), p = exp(0.2 a_src), q = exp(0.2 a_dst).
With adjH = adj * [s>=0] (one compare + one multiply per element, no exp
on the LxL grid) and rv = v*[h|1], rq = q*[h|1]:
  out_aug = p .* (adjT@rq + adjH@(-rq)) + u .* (adjH@rv)
where the first parenthesis accumulates in one PSUM region (free subtract)
and col 64 of the aug matmuls is the softmax denominator.

Each core handles one batch sample (B=8 across 8 cores).
"""

import numpy as np

import concourse.bass as bass
import concourse.mybir as mybir
import concourse.tile as tile
from concourse import bacc
from concourse.bass_utils import run_bass_kernel_spmd
from concourse.masks import make_identity

P = 128
B, L, CIN, COUT, HEADS = 8, 2048, 256, 256, 4
HD = COUT // HEADS          # 64
NT = L // P                 # 16 tiles along L
KB = CIN // P               # 2 chunks along cin/cout
NEG = 0.2
N_CORES = 8

F32 = mybir.dt.float32
BF16 = mybir.dt.bfloat16
U8 = mybir.dt.uint8
F8 = mybir.dt.float8e4
AF = mybir.ActivationFunctionType
OP = mybir.AluOpType

_NC_CACHE = {}


def _build():
    nc = bacc.Bacc(None, target_bir_lowering=False, debug=False)
    x_in = nc.declare_dram_parameter("x", [L, CIN], F32, isOutput=False)
    adj_in = nc.declare_dram_parameter("adj", [L, L], U8, isOutput=False)
    w_in = nc.declare_dram_parameter("W", [COUT, CIN], F32, isOutput=False)
    asrc_in = nc.declare_dram_parameter("att_src", [1, HEADS, 1, HD], F32, isOutput=False)
    adst_in = nc.declare_dram_parameter("att_dst", [1, HEADS, 1, HD], F32, isOutput=False)
    out_d = nc.declare_dram_parameter("out", [L, COUT], F32, isOutput=True)

    with tile.TileContext(nc) as tc:
        with (
            tc.tile_pool(name="const", bufs=1) as cpool,
            tc.tile_pool(name="big", bufs=1) as big,
        ):
            ident_f8 = cpool.tile([P, P], F8)
            make_identity(nc, ident_f8)
            ident_f32 = cpool.tile([P, P], F32)
            make_identity(nc, ident_f32)
            ones_bf = cpool.tile([1, P], BF16)
            nc.vector.memset(ones_bf[:], 1.0)

            # persistent big tensors
            adjT = big.tile([P, NT, L], BF16)          # adj transposed, j on partitions
            xT_bf = big.tile([P, KB, L], BF16)
            h_bf = big.tile([P, NT, COUT], BF16)       # h natural (L on partitions)
            wT_bf = big.tile([P, KB, COUT], BF16)      # W^T (cin on partitions)
            a_bf = big.tile([8, L], BF16)
            a_cols = big.tile([P, NT, 8], F32)         # transposed score columns

            # ---------------- prep: mask transpose + x/W/scores/h ----------------
            with (
                tc.tile_pool(name="adj_nat", bufs=3) as anat_pool,
                tc.tile_pool(name="xload", bufs=3) as xload,
                tc.tile_pool(name="big2", bufs=1) as big2,
                tc.tile_pool(name="adj_ps", bufs=2, space="PSUM") as aps_pool,
                tc.tile_pool(name="prep_ps", bufs=2, space="PSUM") as pps,
                tc.tile_pool(name="small_ps", bufs=2, space="PSUM") as sps,
            ):
                xT_f32 = big2.tile([P, KB, L], F32)        # x^T (cin on partitions)
                w_nat = big2.tile([P, KB, CIN], F32)       # W natural (cout on partitions)
                attW = big2.tile([P, KB, 2 * HEADS], F32)  # [cout, 2H] att matrix
                attc = big2.tile([P, KB, 2 * HEADS], F32)  # (W^T @ attW): [cin, 2H]
                a_all = big2.tile([8, L], F32)             # 2H score rows

                # W natural + attW (DMA only, early)
                nc.sync.dma_start(
                    out=w_nat[:], in_=w_in[:].rearrange("(kb p) c -> p kb c", p=P)
                )
                nc.vector.memset(attW[:], 0.0)
                for h in range(HEADS):
                    cb, prow = divmod(HD * h, P)
                    nc.sync.dma_start(
                        out=attW[prow : prow + HD, cb, 2 * h : 2 * h + 1],
                        in_=asrc_in[0, h, 0, :].rearrange("(d one) -> d one", one=1),
                    )
                    nc.sync.dma_start(
                        out=attW[prow : prow + HD, cb, 2 * h + 1 : 2 * h + 2],
                        in_=adst_in[0, h, 0, :].rearrange("(d one) -> d one", one=1),
                    )

                # x^T first (feeds the score path)
                for c in range(NT):
                    xn = xload.tile([P, CIN], F32)
                    nc.sync.dma_start(out=xn[:], in_=x_in[c * P : (c + 1) * P, :])
                    xp = pps.tile([P, KB, P], F32, tag="prep")
                    for kb in range(KB):
                        nc.tensor.transpose(
                            xp[:, kb, :], xn[:, kb * P : (kb + 1) * P], ident_f32[:]
                        )
                    nc.scalar.activation(
                        xT_f32[:, :, c * P : (c + 1) * P], xp[:], AF.Copy, bias=0.0, scale=1.0
                    )

                def adj_tile(c):
                    an = anat_pool.tile([P, L], U8, name="an")
                    nc.sync.dma_start(out=an[:], in_=adj_in[c * P : (c + 1) * P, :])
                    an_f8 = an[:].bitcast(F8)
                    tp = aps_pool.tile([P, NT, P, 2], F8, tag="tp", name="tp")
                    for t in range(NT):
                        nc.tensor.transpose(
                            tp[:, t, :, 0], an_f8[:, t * P : (t + 1) * P], ident_f8[:]
                        )
                    # fp8 0x01 = 2^-9; scale 512 -> exact 1.0 in bf16.
                    # Alternate evacuation between ACT and DVE to halve the pacer.
                    if c % 2 == 0:
                        nc.scalar.activation(
                            adjT[:, :, c * P : (c + 1) * P], tp[:, :, :, 0],
                            AF.Copy, bias=0.0, scale=512.0,
                        )
                    else:
                        nc.vector.tensor_scalar(
                            out=adjT[:, :, c * P : (c + 1) * P], in0=tp[:, :, :, 0],
                            scalar1=512.0, scalar2=None, op0=OP.mult,
                        )

                # score path (feeds head-0 build)
                for mb in range(KB):
                    ap_ps = sps.tile([P, 2 * HEADS], F32, tag="small")
                    for cb in range(KB):
                        nc.tensor.matmul(
                            ap_ps[:], w_nat[:, cb, mb * P : (mb + 1) * P], attW[:, cb, :],
                            start=(cb == 0), stop=(cb == KB - 1),
                        )
                    nc.scalar.activation(attc[:, mb, :], ap_ps[:], AF.Copy, bias=0.0, scale=1.0)

                for nb in range(4):
                    a_ps = sps.tile([8, 512], F32, tag="small")
                    for kb in range(KB):
                        nc.tensor.matmul(
                            a_ps[:], attc[:, kb, :], xT_f32[:, kb, nb * 512 : (nb + 1) * 512],
                            start=(kb == 0), stop=(kb == KB - 1),
                        )
                    nc.scalar.activation(
                        a_all[:, nb * 512 : (nb + 1) * 512], a_ps[:], AF.Copy, bias=0.0, scale=1.0
                    )
                nc.vector.tensor_copy(a_bf[:], a_all[:])

                for t in range(NT):
                    acp = sps.tile([P, 8], F32, tag="small")
                    nc.tensor.transpose(
                        acp[:], a_all[0:8, t * P : (t + 1) * P], ident_f32[0:8, 0:8]
                    )
                    nc.scalar.activation(a_cols[:, t, :], acp[:], AF.Copy, bias=0.0, scale=1.0)

                # adjacency tiles (first half feeds head-0 half-0 build)
                for c in range(NT):
                    adj_tile(c)

                # W^T then h = x @ W.T (after the score path)
                nc.vector.tensor_copy(xT_bf[:], xT_f32[:])
                for cb in range(KB):
                    wp = pps.tile([P, KB, P], F32, tag="prep")
                    for ib in range(KB):
                        nc.tensor.transpose(
                            wp[:, ib, :], w_nat[:, cb, ib * P : (ib + 1) * P], ident_f32[:]
                        )
                    for ib in range(KB):
                        nc.scalar.activation(
                            wT_bf[:, ib, cb * P : (cb + 1) * P], wp[:, ib, :],
                            AF.Copy, bias=0.0, scale=1.0,
                        )
                for c in range(NT):
                    hp = pps.tile([P, COUT], F32, tag="prep")
                    for kb in range(KB):
                        nc.tensor.matmul(
                            hp[:], xT_bf[:, kb, c * P : (c + 1) * P], wT_bf[:, kb, :],
                            start=(kb == 0), stop=(kb == KB - 1),
                        )
                    nc.scalar.activation(h_bf[:, c, :], hp[:], AF.Copy, bias=0.0, scale=1.0)

            # ---------------- stage 3: per-head attention ----------------
            # Software-pipelined: emit DVE builds for head h while PE computes
            # head h-1, so DVE never head-of-line-blocks behind epilogue ops.
            NAUG = HD + 1
            HALF = L // 2
            with (
                tc.tile_pool(name="cols", bufs=2) as colp,
                tc.tile_pool(name="rhs", bufs=2) as rhsp,
                tc.tile_pool(name="bc", bufs=2) as bcp,
                tc.tile_pool(name="adjH", bufs=2) as adjHp,
                tc.tile_pool(name="sig", bufs=4) as sigp,
                tc.tile_pool(name="est", bufs=2) as estp,
                tc.tile_pool(name="outst", bufs=2) as outp,
                tc.tile_pool(name="mm_ps", bufs=3, space="PSUM") as mmps,
                tc.tile_pool(name="bc_ps", bufs=1, space="PSUM") as bcps,
            ):

                def build_head(h):
                    st = {}
                    ucol = st["ucol"] = colp.tile([P, NT], F32, tag="ucol", name="ucol")
                    pcol = st["pcol"] = colp.tile([P, NT], F32, tag="pcol", name="pcol")
                    vcol = st["vcol"] = colp.tile([P, NT], F32, tag="vcol", name="vcol")
                    qcol = st["qcol"] = colp.tile([P, NT], F32, tag="qcol", name="qcol")
                    nadst = st["nadst"] = colp.tile([P, NT], F32, tag="nadst", name="nadst")
                    asl = a_cols[:, :, 2 * h : 2 * h + 1].rearrange("p t one -> p (t one)")
                    adl = a_cols[:, :, 2 * h + 1 : 2 * h + 2].rearrange("p t one -> p (t one)")
                    nc.scalar.activation(ucol[:], asl, AF.Exp, bias=0.0, scale=1.0)
                    nc.scalar.activation(pcol[:], asl, AF.Exp, bias=0.0, scale=NEG)
                    nc.scalar.activation(vcol[:], adl, AF.Exp, bias=0.0, scale=1.0)
                    nc.scalar.activation(qcol[:], adl, AF.Exp, bias=0.0, scale=NEG)
                    nc.vector.tensor_scalar(
                        out=nadst[:], in0=adl, scalar1=-1.0, scalar2=None, op0=OP.mult
                    )

                    # a_src broadcast tile [128, L] bf16
                    arow = bcp.tile([1, L], BF16, tag="arow")
                    nc.sync.dma_start(out=arow[:], in_=a_bf[2 * h : 2 * h + 1, :])
                    bcast = st["bcast"] = bcp.tile([P, L], BF16, tag="bcast", name="bcast")
                    for nb in range(4):
                        bps = bcps.tile([P, 512], F32, tag="bps")
                        nc.tensor.matmul(
                            bps[:], ones_bf[:], arow[0:1, nb * 512 : (nb + 1) * 512],
                            start=True, stop=True,
                        )
                        nc.scalar.activation(
                            bcast[:, nb * 512 : (nb + 1) * 512], bps[:], AF.Copy, bias=0.0, scale=1.0
                        )

                    # rhs tensors: rq = q*[h|1]; rnv = [-q*[h|1] | v*[h|1]] packed
                    nqcol = colp.tile([P, NT], F32, tag="nqcol", name="nqcol")
                    nc.vector.tensor_scalar(
                        out=nqcol[:], in0=qcol[:], scalar1=-1.0, scalar2=None, op0=OP.mult
                    )
                    rq = st["rq"] = rhsp.tile([P, NT, NAUG], BF16, tag="rq", name="rq")
                    rnv = st["rnv"] = rhsp.tile([P, NT, 2 * NAUG], BF16, tag="rnv", name="rnv")
                    for t in range(NT):
                        hsrc = h_bf[:, t, h * HD : (h + 1) * HD]
                        nc.scalar.activation(
                            rq[:, t, 0:HD], hsrc, AF.Identity, bias=0.0,
                            scale=qcol[:, t : t + 1],
                        )
                        nc.scalar.activation(
                            rnv[:, t, 0:HD], hsrc, AF.Identity, bias=0.0,
                            scale=nqcol[:, t : t + 1],
                        )
                        nc.scalar.activation(
                            rnv[:, t, NAUG : NAUG + HD], hsrc, AF.Identity, bias=0.0,
                            scale=vcol[:, t : t + 1],
                        )
                    nc.vector.tensor_copy(
                        rq[:, :, HD : HD + 1].rearrange("p t one -> p (t one)"), qcol[:]
                    )
                    nc.vector.tensor_copy(
                        rnv[:, :, HD : HD + 1].rearrange("p t one -> p (t one)"), nqcol[:]
                    )
                    nc.vector.tensor_copy(
                        rnv[:, :, 2 * NAUG - 1 : 2 * NAUG].rearrange("p t one -> p (t one)"), vcol[:]
                    )

                    # adjH = adjT * [s >= 0], both halves (separate tiles so the
                    # compute phase of half 0 doesn't wait on half 1's build)
                    st["adjH"] = []
                    for half in range(2):
                        i0 = half * HALF
                        adjH = adjHp.tile([P, NT, HALF], BF16, tag="adjH", name="adjH")
                        st["adjH"].append(adjH)
                        for t in range(NT):
                            sg = sigp.tile([P, HALF], BF16, tag="sig")
                            nc.vector.tensor_scalar(
                                out=sg[:], in0=bcast[:, i0 : i0 + HALF],
                                scalar1=nadst[:, t : t + 1], scalar2=None, op0=OP.is_ge,
                            )
                            nc.vector.tensor_tensor(
                                out=adjH[:, t, :], in0=adjT[:, t, i0 : i0 + HALF],
                                in1=sg[:], op=OP.mult,
                            )
                    return st

                def compute_head(h, st):
                    ucol, pcol = st["ucol"], st["pcol"]
                    rq, rnv = st["rq"], st["rnv"]
                    out_stage = outp.tile([P, NT, HD], F32, tag="outst")
                    s_all = estp.tile([P, NT, NAUG + 1], F32, tag="s_all")
                    for half in range(2):
                        i0 = half * HALF
                        adjH = st["adjH"][half]
                        for ic in range(HALF // P):
                            cg = half * (HALF // P) + ic
                            isl = slice(i0 + ic * P, i0 + (ic + 1) * P)
                            hsl = slice(ic * P, (ic + 1) * P)
                            po = mmps.tile([P, 2 * NAUG], F32, tag="po", bufs=7)
                            for t in range(NT):
                                # adjH @ [-rq | rv] -> cols 0:130 (first starts the bank)
                                nc.tensor.matmul(
                                    po[:], adjH[:, t, hsl], rnv[:, t, :],
                                    start=(t == 0), stop=(t == NT - 1),
                                    skip_group_check=True,
                                )
                                # adjT @ rq accumulates into cols 0:65
                                nc.tensor.matmul(
                                    po[:, 0:NAUG], adjT[:, t, isl], rq[:, t, :],
                                    start=False, stop=(t == NT - 1),
                                    skip_group_check=True,
                                )
                            e1 = estp.tile([P, 2, NAUG], F32, tag="e1", bufs=4)
                            nc.scalar.activation(
                                e1[:, 0, :], po[:, 0:NAUG], AF.Identity,
                                bias=0.0, scale=pcol[:, cg : cg + 1],
                            )
                            nc.scalar.activation(
                                e1[:, 1, :], po[:, NAUG : 2 * NAUG], AF.Identity,
                                bias=0.0, scale=ucol[:, cg : cg + 1],
                            )
                            nc.gpsimd.tensor_tensor(
                                out=s_all[:, cg, 0:NAUG], in0=e1[:, 0, :], in1=e1[:, 1, :], op=OP.add
                            )

                    rall = estp.tile([P, NT], F32, tag="rall")
                    nc.vector.reciprocal(
                        rall[:], s_all[:, :, HD : HD + 1].rearrange("p t one -> p (t one)")
                    )
                    for cg in range(NT):
                        nc.vector.tensor_scalar(
                            out=out_stage[:, cg, :], in0=s_all[:, cg, 0:HD],
                            scalar1=rall[:, cg : cg + 1], scalar2=None, op0=OP.mult,
                        )
                    nc.gpsimd.dma_start(
                        out=out_d[:].rearrange("(c p) (hh d) -> p c hh d", p=P, d=HD)[:, :, h, :],
                        in_=out_stage[:],
                    )

                prev = None
                for h in range(HEADS):
                    st = build_head(h)
                    if prev is not None:
                        compute_head(h - 1, prev)
                    prev = st
                compute_head(HEADS - 1, prev)

    nc.finalize()
    return nc


def kernel(x, adj_mask, W, att_src, att_dst):
    if "nc" not in _NC_CACHE:
        _NC_CACHE["nc"] = _build()
    nc = _NC_CACHE["nc"]

    x = np.ascontiguousarray(np.asarray(x, dtype=np.float32))
    W = np.ascontiguousarray(np.asarray(W, dtype=np.float32))
    att_src = np.ascontiguousarray(np.asarray(att_src, dtype=np.float32))
    att_dst = np.ascontiguousarray(np.asarray(att_dst, dtype=np.float32))
    adj = np.ascontiguousarray(adj_mask).view(np.uint8)

    in_maps = [
        {
            "x": x[b],
            "adj": adj[b],
            "W": W,
            "att_src": att_src,
            "att_dst": att_dst,
        }
        for b in range(N_CORES)
    ]
    res = run_bass_kernel_spmd(nc, in_maps, core_ids=list(range(N_CORES)))
    out = np.stack([res.results[b]["out"] for b in range(N_CORES)], axis=0)
    return out.astype(np.float32)
